# revision 3
# baseline (speedup 1.0000x reference)
"""Additive (Bahdanau) attention kernel for Trainium2, 8 NeuronCores.

Problem shapes (hardcoded): B=8, TQ=128, TV=256, D=512, U=256.
Sharding: data-parallel over batch B -> one batch element per core.

Per-core algorithm (all on-chip after the initial DMAs):
  w1vT[u,v]  = (values @ W1)^T           via PE (K=d chunks)
  w2qT[u,q]  = (query  @ W2)^T + (b1+b2) via PE + ACT bias
  for each block of 16 q:
      pre[u,(q,c,v)] = w1vT[u,(c,v)] + w2qT[u,(c,q)]
        (13 q: DVE broadcast add; 1 q: GPSIMD; 2 q: fused into ACT tanh bias)
      feat = tanh(pre)   (ACT, one big-FD instr + the 2 fused-bias q's)
      score pairs: for q-pair p, chunk c: one N=512 matmul with V placed at
        window columns 2p,2p+1 -> even q's score in psum cols 0:256, odd in
        256:512; the unread half of each row accumulates don't-care values.
  attn = exp(score) (no max-sub needed; |score| <= sum|V| ~ 13), with an
  even/odd predicated select; rowsum on DVE; context = attnT^T @ values
  scaled by 1/rowsum.  bv is dropped: softmax is shift-invariant.
"""
import sys
import numpy as np

if '/opt/trn_rl_repo' not in sys.path:
    sys.path.insert(0, '/opt/trn_rl_repo')

B, TQ, TV, D, U = 8, 128, 256, 512, 256
P = 128          # partitions
KD = D // P      # 4 k-chunks over d
CU = U // P      # 2 chunks over u
CV = TV // P     # 2 chunks over v
BQ = 16          # q-block size
NBLK = TQ // BQ  # 8 blocks
N_GP = 1         # q's per block added on GPSIMD
N_ACT = 2        # q's per block fused into ACT (tanh with bias)
N_DVE = BQ - N_GP - N_ACT

_compiled = None


def _build():
    import concourse.bass as bass
    import concourse.tile as tile
    from concourse import bacc, mybir

    f32 = mybir.dt.float32
    AF = mybir.ActivationFunctionType

    nc = bacc.Bacc("TRN2", target_bir_lowering=False, debug=False,
                   enable_asserts=True, num_devices=B)

    W1_d = nc.dram_tensor("W1", [D, U], f32, kind="ExternalInput").ap()
    W2_d = nc.dram_tensor("W2", [D, U], f32, kind="ExternalInput").ap()
    QT_d = nc.dram_tensor("QT", [D, TQ], f32, kind="ExternalInput").ap()
    VT_d = nc.dram_tensor("VT", [D, TV], f32, kind="ExternalInput").ap()
    VAL_d = nc.dram_tensor("VAL", [TV, D], f32, kind="ExternalInput").ap()
    VW_d = nc.dram_tensor("VW", [P, CU, 256], f32, kind="ExternalInput").ap()
    B12_d = nc.dram_tensor("B12", [P, CU], f32, kind="ExternalInput").ap()
    ID_d = nc.dram_tensor("ID", [P, P], f32, kind="ExternalInput").ap()
    ME_d = nc.dram_tensor("ME", [P, 1], mybir.dt.uint8, kind="ExternalInput").ap()
    OUT_d = nc.dram_tensor("OUT", [TQ, D], f32, kind="ExternalOutput").ap()

    with tile.TileContext(nc) as tc:
        with (
            tc.tile_pool(name="cst", bufs=1) as cst,
            tc.tile_pool(name="pre_p", bufs=3) as pre_p,
            tc.tile_pool(name="sm", bufs=1) as sm,
            tc.tile_pool(name="ps", bufs=1, space=bass.MemorySpace.PSUM) as ps,
        ):
            # ---- constant / input tiles ----
            w1 = cst.tile([P, KD, U], f32, tag="w1")
            nc.sync.dma_start(w1[:], W1_d.rearrange("(k p) u -> p k u", p=P))
            w2 = cst.tile([P, KD, U], f32, tag="w2")
            nc.sync.dma_start(w2[:], W2_d.rearrange("(k p) u -> p k u", p=P))
            qt = cst.tile([P, KD, TQ], f32, tag="qt")
            nc.sync.dma_start(qt[:], QT_d.rearrange("(k p) q -> p k q", p=P))
            vt = cst.tile([P, KD, TV], f32, tag="vt")
            nc.sync.dma_start(vt[:], VT_d.rearrange("(k p) v -> p k v", p=P))
            val = cst.tile([P, CV, D], f32, tag="val")
            nc.sync.dma_start(val[:], VAL_d.rearrange("(c p) d -> p c d", p=P))
            vw = cst.tile([P, CU, 256], f32, tag="vw")
            nc.sync.dma_start(vw[:], VW_d)
            b12 = cst.tile([P, CU], f32, tag="b12")
            nc.sync.dma_start(b12[:], B12_d)
            idt = cst.tile([P, P], f32, tag="idt")
            nc.sync.dma_start(idt[:], ID_d)
            mev = cst.tile([P, 1], mybir.dt.uint8, tag="mev")
            nc.sync.dma_start(mev[:], ME_d)

            # ---- projections ----
            psW1 = ps.tile([P, CU, TV], f32, tag="psW1")   # one bank
            for c in range(CU):
                for k in range(KD):
                    nc.tensor.matmul(psW1[:, c, :],
                                     w1[:, k, c * P:(c + 1) * P],
                                     vt[:, k, :],
                                     start=(k == 0), stop=(k == KD - 1))
            w1vT = cst.tile([P, CU, TV], f32, tag="w1vT")
            nc.scalar.copy(w1vT[:], psW1[:])

            psW2 = ps.tile([P, CU, TQ], f32, tag="psW2")   # half bank
            for c in range(CU):
                for k in range(KD):
                    nc.tensor.matmul(psW2[:, c, :],
                                     w2[:, k, c * P:(c + 1) * P],
                                     qt[:, k, :],
                                     start=(k == 0), stop=(k == KD - 1))
            w2qT = cst.tile([P, CU, TQ], f32, tag="w2qT")
            for c in range(CU):
                nc.scalar.activation(w2qT[:, c, :], psW2[:, c, :],
                                     AF.Identity, bias=b12[:, c:c + 1])

            # ---- score phase ----
            score_ps = ps.tile([P, 2 * TV], f32, tag="score")  # one bank
            n_mm = (TQ // 2) * CU
            mm = 0
            for blk in range(NBLK):
                q0 = blk * BQ
                pre = pre_p.tile([P, BQ, CU, TV], f32, tag="pre")
                # DVE adds for q0 .. q0+N_DVE-1
                in0 = w1vT[:].unsqueeze(1).broadcast_to([P, N_DVE, CU, TV])
                in1 = (w2qT[:, :, q0:q0 + N_DVE]
                       .rearrange("p c q -> p q c")
                       .unsqueeze(3).broadcast_to([P, N_DVE, CU, TV]))
                nc.vector.tensor_add(pre[:, 0:N_DVE, :, :], in0, in1)
                # GPSIMD adds (per (q,c) tensor_scalar_add)
                for j in range(N_GP):
                    ql = N_DVE + j
                    q = q0 + ql
                    for c in range(CU):
                        nc.gpsimd.tensor_scalar_add(pre[:, ql, c, :],
                                                    w1vT[:, c, :],
                                                    w2qT[:, c, q:q + 1])
                # big tanh over the DVE+GPSIMD q's
                nt = N_DVE + N_GP
                nc.scalar.activation(pre[:, 0:nt, :, :], pre[:, 0:nt, :, :],
                                     AF.Tanh)
                # ACT-fused q's: feat = tanh(w1vT + w2q_col) directly
                for j in range(N_ACT):
                    ql = nt + j
                    q = q0 + ql
                    for c in range(CU):
                        nc.scalar.activation(pre[:, ql, c, :], w1vT[:, c, :],
                                             AF.Tanh,
                                             bias=w2qT[:, c, q:q + 1])
                # score pair matmuls: pair p covers q=2p, 2p+1
                for pl in range(BQ // 2):
                    q = q0 + 2 * pl
                    for c in range(CU):
                        nc.tensor.matmul(score_ps[:],
                                         vw[:, c, 127 - q:255 - q],
                                         pre[:, 2 * pl:2 * pl + 2, c, :],
                                         start=(mm == 0), stop=(mm == n_mm - 1))
                        mm += 1

            # ---- softmax (no max subtraction; scores are bounded) ----
            att_e = sm.tile([P, TV], f32, tag="att_e")
            att_o = sm.tile([P, TV], f32, tag="att_o")
            nc.scalar.activation(att_e[:], score_ps[:, 0:TV], AF.Exp)
            nc.scalar.activation(att_o[:], score_ps[:, TV:2 * TV], AF.Exp)
            att = sm.tile([P, TV], f32, tag="att")
            nc.vector.tensor_copy(att[:], att_o[:])
            nc.vector.copy_predicated(att[:], mev[:].broadcast_to([P, TV]),
                                      att_e[:])
            sums = sm.tile([P, 2], f32, tag="sums")
            nc.vector.reduce_sum(sums[:, 0:1], att[:],
                                 axis=mybir.AxisListType.X)
            nc.vector.reciprocal(sums[:, 1:2], sums[:, 0:1])

            # ---- context = softmax(score) @ values ----
            psT = ps.tile([P, CV, P], f32, tag="psT")      # half bank
            for c in range(CV):
                nc.tensor.transpose(psT[:, c, :], att[:, c * P:(c + 1) * P],
                                    idt[:])
            attnT = sm.tile([P, CV, P], f32, tag="attnT")
            nc.vector.tensor_copy(attnT[:], psT[:])

            ctx_ps = ps.tile([P, D], f32, tag="ctx")       # one bank
            for c in range(CV):
                nc.tensor.matmul(ctx_ps[:], attnT[:, c, :], val[:, c, :],
                                 start=(c == 0), stop=(c == CV - 1))
            ctx = sm.tile([P, D], f32, tag="ctxsb")
            nc.vector.tensor_scalar_mul(ctx[:], ctx_ps[:], sums[:, 1:2])
            nc.sync.dma_start(OUT_d, ctx[:])

    nc.compile()
    return nc


def _prep_shared(W1, b1, W2, b2, V, bv):
    Vw = np.zeros((P, CU, 256), np.float32)
    for c in range(CU):
        Vw[:, c, 127] = V[c * P:(c + 1) * P, 0]
        Vw[:, c, 128] = V[c * P:(c + 1) * P, 0]
    b12 = (b1 + b2).astype(np.float32).reshape(CU, P).T.copy()
    ident = np.eye(P, dtype=np.float32)
    maskE = (1 - (np.arange(P) % 2)).astype(np.uint8).reshape(P, 1)
    return {
        "W1": np.ascontiguousarray(W1, np.float32),
        "W2": np.ascontiguousarray(W2, np.float32),
        "VW": Vw,
        "B12": np.ascontiguousarray(b12),
        "ID": ident,
        "ME": maskE,
    }


def kernel(query, values, W1, b1, W2, b2, V, bv, _trace=False, _tmpdir=None):
    global _compiled
    from concourse.bass_utils import run_bass_kernel_spmd

    query = np.asarray(query, np.float32)
    values = np.asarray(values, np.float32)
    shared = _prep_shared(np.asarray(W1), np.asarray(b1), np.asarray(W2),
                          np.asarray(b2), np.asarray(V), np.asarray(bv))

    if _compiled is None:
        _compiled = _build()
    nc = _compiled

    in_maps = []
    for i in range(B):
        m = dict(shared)
        m["QT"] = np.ascontiguousarray(query[i].T)
        m["VT"] = np.ascontiguousarray(values[i].T)
        m["VAL"] = np.ascontiguousarray(values[i])
        in_maps.append(m)

    kw = {}
    if _trace:
        kw.update(trace=True, tmpdir=_tmpdir)
    res = run_bass_kernel_spmd(nc, in_maps, core_ids=list(range(B)), **kw)
    out = np.stack([res.results[i]["OUT"] for i in range(B)], axis=0)
    if _trace:
        kernel._last_trace = res
    return out


# revision 4
# speedup vs baseline: 1.0940x; 1.0940x over previous
"""Additive (Bahdanau) attention kernel for Trainium2, 8 NeuronCores.

Problem shapes (hardcoded): B=8, TQ=128, TV=256, D=512, U=256.
Sharding: data-parallel over batch B -> one batch element per core.

Per-core algorithm (all on-chip after the initial DMAs):
  w1vT[u,v]  = (values @ W1)^T           via PE (K=d chunks)
  w2qT[u,q]  = (query  @ W2)^T + (b1+b2) via PE + ACT bias
  for each block of 16 q:
      pre[u,(q,c,v)] = w1vT[u,(c,v)] + w2qT[u,(c,q)]
        (13 q: DVE broadcast add; 1 q: GPSIMD; 2 q: fused into ACT tanh bias)
      feat = tanh(pre)   (ACT, one big-FD instr + the 2 fused-bias q's)
      score pairs: for q-pair p, chunk c: one N=512 matmul with V placed at
        window columns 2p,2p+1 -> even q's score in psum cols 0:256, odd in
        256:512; the unread half of each row accumulates don't-care values.
  attn = exp(score) (no max-sub needed; |score| <= sum|V| ~ 13), with an
  even/odd predicated select; rowsum on DVE; context = attnT^T @ values
  scaled by 1/rowsum.  bv is dropped: softmax is shift-invariant.
"""
import sys
import numpy as np

if '/opt/trn_rl_repo' not in sys.path:
    sys.path.insert(0, '/opt/trn_rl_repo')

B, TQ, TV, D, U = 8, 128, 256, 512, 256
P = 128          # partitions
KD = D // P      # 4 k-chunks over d
CU = U // P      # 2 chunks over u
CV = TV // P     # 2 chunks over v
BQ = 16          # q-block size
NBLK = TQ // BQ  # 8 blocks
N_GP = 0         # q's per block added on GPSIMD (gpsimd tensor ops are ~7us each: unusable)
N_ACT = 2        # q's per block fused into ACT (tanh with bias)
N_DVE = BQ - N_GP - N_ACT

_compiled = None


def _build():
    import concourse.bass as bass
    import concourse.tile as tile
    from concourse import bacc, mybir

    f32 = mybir.dt.float32
    AF = mybir.ActivationFunctionType

    nc = bacc.Bacc("TRN2", target_bir_lowering=False, debug=False,
                   enable_asserts=True, num_devices=B)

    W1_d = nc.dram_tensor("W1", [D, U], f32, kind="ExternalInput").ap()
    W2_d = nc.dram_tensor("W2", [D, U], f32, kind="ExternalInput").ap()
    QT_d = nc.dram_tensor("QT", [D, TQ], f32, kind="ExternalInput").ap()
    VT_d = nc.dram_tensor("VT", [D, TV], f32, kind="ExternalInput").ap()
    VAL_d = nc.dram_tensor("VAL", [TV, D], f32, kind="ExternalInput").ap()
    VW_d = nc.dram_tensor("VW", [P, CU, 256], f32, kind="ExternalInput").ap()
    B12_d = nc.dram_tensor("B12", [P, CU], f32, kind="ExternalInput").ap()
    ID_d = nc.dram_tensor("ID", [P, P], f32, kind="ExternalInput").ap()
    ME_d = nc.dram_tensor("ME", [P, 1], mybir.dt.uint8, kind="ExternalInput").ap()
    OUT_d = nc.dram_tensor("OUT", [TQ, D], f32, kind="ExternalOutput").ap()

    with tile.TileContext(nc) as tc:
        with (
            tc.tile_pool(name="cst", bufs=1) as cst,
            tc.tile_pool(name="pre_p", bufs=3) as pre_p,
            tc.tile_pool(name="sm", bufs=1) as sm,
            tc.tile_pool(name="ps", bufs=1, space=bass.MemorySpace.PSUM) as ps,
        ):
            # ---- constant / input tiles ----
            w1 = cst.tile([P, KD, U], f32, tag="w1")
            nc.sync.dma_start(w1[:], W1_d.rearrange("(k p) u -> p k u", p=P))
            w2 = cst.tile([P, KD, U], f32, tag="w2")
            nc.sync.dma_start(w2[:], W2_d.rearrange("(k p) u -> p k u", p=P))
            qt = cst.tile([P, KD, TQ], f32, tag="qt")
            nc.sync.dma_start(qt[:], QT_d.rearrange("(k p) q -> p k q", p=P))
            vt = cst.tile([P, KD, TV], f32, tag="vt")
            nc.sync.dma_start(vt[:], VT_d.rearrange("(k p) v -> p k v", p=P))
            val = cst.tile([P, CV, D], f32, tag="val")
            nc.sync.dma_start(val[:], VAL_d.rearrange("(c p) d -> p c d", p=P))
            vw = cst.tile([P, CU, 256], f32, tag="vw")
            nc.sync.dma_start(vw[:], VW_d)
            b12 = cst.tile([P, CU], f32, tag="b12")
            nc.sync.dma_start(b12[:], B12_d)
            idt = cst.tile([P, P], f32, tag="idt")
            nc.sync.dma_start(idt[:], ID_d)
            mev = cst.tile([P, 1], mybir.dt.uint8, tag="mev")
            nc.sync.dma_start(mev[:], ME_d)

            # ---- projections ----
            psW1 = ps.tile([P, CU, TV], f32, tag="psW1")   # one bank
            for c in range(CU):
                for k in range(KD):
                    nc.tensor.matmul(psW1[:, c, :],
                                     w1[:, k, c * P:(c + 1) * P],
                                     vt[:, k, :],
                                     start=(k == 0), stop=(k == KD - 1))
            w1vT = cst.tile([P, CU, TV], f32, tag="w1vT")
            nc.scalar.copy(w1vT[:], psW1[:])

            psW2 = ps.tile([P, CU, TQ], f32, tag="psW2")   # half bank
            for c in range(CU):
                for k in range(KD):
                    nc.tensor.matmul(psW2[:, c, :],
                                     w2[:, k, c * P:(c + 1) * P],
                                     qt[:, k, :],
                                     start=(k == 0), stop=(k == KD - 1))
            w2qT = cst.tile([P, CU, TQ], f32, tag="w2qT")
            for c in range(CU):
                nc.scalar.activation(w2qT[:, c, :], psW2[:, c, :],
                                     AF.Identity, bias=b12[:, c:c + 1])

            # ---- score phase ----
            score_ps = ps.tile([P, 2 * TV], f32, tag="score")  # one bank
            n_mm = (TQ // 2) * CU
            mm = 0
            for blk in range(NBLK):
                q0 = blk * BQ
                # layout [P, CU, BQ, TV]: keeps each pair's matmul rhs
                # (2 q's x 256 v within one chunk) contiguous
                pre = pre_p.tile([P, CU, BQ, TV], f32, tag="pre")
                # DVE adds for q0 .. q0+N_DVE-1
                in0 = w1vT[:].unsqueeze(2).broadcast_to([P, CU, N_DVE, TV])
                in1 = (w2qT[:, :, q0:q0 + N_DVE]
                       .unsqueeze(3).broadcast_to([P, CU, N_DVE, TV]))
                nc.vector.tensor_add(pre[:, :, 0:N_DVE, :], in0, in1)
                # GPSIMD adds (per (q,c) tensor_scalar_add)
                for j in range(N_GP):
                    ql = N_DVE + j
                    q = q0 + ql
                    for c in range(CU):
                        nc.gpsimd.tensor_scalar_add(pre[:, c, ql, :],
                                                    w1vT[:, c, :],
                                                    w2qT[:, c, q:q + 1])
                # big tanh over the DVE+GPSIMD q's
                nt = N_DVE + N_GP
                nc.scalar.activation(pre[:, :, 0:nt, :], pre[:, :, 0:nt, :],
                                     AF.Tanh)
                # ACT-fused q's: feat = tanh(w1vT + w2q_col) directly
                for j in range(N_ACT):
                    ql = nt + j
                    q = q0 + ql
                    for c in range(CU):
                        nc.scalar.activation(pre[:, c, ql, :], w1vT[:, c, :],
                                             AF.Tanh,
                                             bias=w2qT[:, c, q:q + 1])
                # score pair matmuls: pair p covers q=2p, 2p+1
                for pl in range(BQ // 2):
                    q = q0 + 2 * pl
                    for c in range(CU):
                        nc.tensor.matmul(score_ps[:],
                                         vw[:, c, 127 - q:255 - q],
                                         pre[:, c, 2 * pl:2 * pl + 2, :],
                                         start=(mm == 0), stop=(mm == n_mm - 1))
                        mm += 1

            # ---- softmax (no max subtraction; scores are bounded) ----
            att_e = sm.tile([P, TV], f32, tag="att_e")
            att_o = sm.tile([P, TV], f32, tag="att_o")
            nc.scalar.activation(att_e[:], score_ps[:, 0:TV], AF.Exp)
            nc.scalar.activation(att_o[:], score_ps[:, TV:2 * TV], AF.Exp)
            att = sm.tile([P, TV], f32, tag="att")
            nc.vector.tensor_copy(att[:], att_o[:])
            nc.vector.copy_predicated(att[:], mev[:].broadcast_to([P, TV]),
                                      att_e[:])
            sums = sm.tile([P, 2], f32, tag="sums")
            nc.vector.reduce_sum(sums[:, 0:1], att[:],
                                 axis=mybir.AxisListType.X)
            nc.vector.reciprocal(sums[:, 1:2], sums[:, 0:1])

            # ---- context = softmax(score) @ values ----
            psT = ps.tile([P, CV, P], f32, tag="psT")      # half bank
            for c in range(CV):
                nc.tensor.transpose(psT[:, c, :], att[:, c * P:(c + 1) * P],
                                    idt[:])
            attnT = sm.tile([P, CV, P], f32, tag="attnT")
            nc.vector.tensor_copy(attnT[:], psT[:])

            ctx_ps = ps.tile([P, D], f32, tag="ctx")       # one bank
            for c in range(CV):
                nc.tensor.matmul(ctx_ps[:], attnT[:, c, :], val[:, c, :],
                                 start=(c == 0), stop=(c == CV - 1))
            ctx = sm.tile([P, D], f32, tag="ctxsb")
            nc.vector.tensor_scalar_mul(ctx[:], ctx_ps[:], sums[:, 1:2])
            nc.sync.dma_start(OUT_d, ctx[:])

    nc.compile()
    return nc


def _prep_shared(W1, b1, W2, b2, V, bv):
    Vw = np.zeros((P, CU, 256), np.float32)
    for c in range(CU):
        Vw[:, c, 127] = V[c * P:(c + 1) * P, 0]
        Vw[:, c, 128] = V[c * P:(c + 1) * P, 0]
    b12 = (b1 + b2).astype(np.float32).reshape(CU, P).T.copy()
    ident = np.eye(P, dtype=np.float32)
    maskE = (1 - (np.arange(P) % 2)).astype(np.uint8).reshape(P, 1)
    return {
        "W1": np.ascontiguousarray(W1, np.float32),
        "W2": np.ascontiguousarray(W2, np.float32),
        "VW": Vw,
        "B12": np.ascontiguousarray(b12),
        "ID": ident,
        "ME": maskE,
    }


def kernel(query, values, W1, b1, W2, b2, V, bv, _trace=False, _tmpdir=None):
    global _compiled
    from concourse.bass_utils import run_bass_kernel_spmd

    query = np.asarray(query, np.float32)
    values = np.asarray(values, np.float32)
    shared = _prep_shared(np.asarray(W1), np.asarray(b1), np.asarray(W2),
                          np.asarray(b2), np.asarray(V), np.asarray(bv))

    if _compiled is None:
        _compiled = _build()
    nc = _compiled

    in_maps = []
    for i in range(B):
        m = dict(shared)
        m["QT"] = np.ascontiguousarray(query[i].T)
        m["VT"] = np.ascontiguousarray(values[i].T)
        m["VAL"] = np.ascontiguousarray(values[i])
        in_maps.append(m)

    kw = {}
    if _trace:
        kw.update(trace=True, tmpdir=_tmpdir)
    res = run_bass_kernel_spmd(nc, in_maps, core_ids=list(range(B)), **kw)
    out = np.stack([res.results[i]["OUT"] for i in range(B)], axis=0)
    if _trace:
        kernel._last_trace = res
    return out


# revision 6
# speedup vs baseline: 1.5344x; 1.4025x over previous
"""Additive (Bahdanau) attention kernel for Trainium2, 8 NeuronCores.

Problem shapes (hardcoded): B=8, TQ=128, TV=256, D=512, U=256.
Sharding: data-parallel over batch B -> one batch element per core.

Per-core algorithm (all on-chip after the initial DMAs):
  w1vT[u,v]  = (values @ W1)^T           via PE (K=d chunks)
  w2qT[u,q]  = (query  @ W2)^T + (b1+b2) via PE + ACT bias
  for each block of 16 q:
      pre[u,(q,c,v)] = w1vT[u,(c,v)] + w2qT[u,(c,q)]
        (13 q: DVE broadcast add; 1 q: GPSIMD; 2 q: fused into ACT tanh bias)
      feat = tanh(pre)   (ACT, one big-FD instr + the 2 fused-bias q's)
      score pairs: for q-pair p, chunk c: one N=512 matmul with V placed at
        window columns 2p,2p+1 -> even q's score in psum cols 0:256, odd in
        256:512; the unread half of each row accumulates don't-care values.
  attn = exp(score) (no max-sub needed; |score| <= sum|V| ~ 13), with an
  even/odd predicated select; rowsum on DVE; context = attnT^T @ values
  scaled by 1/rowsum.  bv is dropped: softmax is shift-invariant.
"""
import sys
import numpy as np

if '/opt/trn_rl_repo' not in sys.path:
    sys.path.insert(0, '/opt/trn_rl_repo')

B, TQ, TV, D, U = 8, 128, 256, 512, 256
P = 128          # partitions
KD = D // P      # 4 k-chunks over d
CU = U // P      # 2 chunks over u
CV = TV // P     # 2 chunks over v
BQ = 16          # q-block size
NBLK = TQ // BQ  # 8 blocks
N_GP = 0         # q's per block added on GPSIMD (gpsimd tensor ops are ~7us each: unusable)
N_ACT = 2        # q's per block fused into ACT (tanh with bias)
N_DVE = BQ - N_GP - N_ACT

_compiled = None


def _build():
    import concourse.bass as bass
    import concourse.tile as tile
    from concourse import bacc, mybir

    f32 = mybir.dt.float32
    AF = mybir.ActivationFunctionType

    nc = bacc.Bacc("TRN2", target_bir_lowering=False, debug=False,
                   enable_asserts=True, num_devices=B)

    W1_d = nc.dram_tensor("W1", [D, U], f32, kind="ExternalInput").ap()
    W2_d = nc.dram_tensor("W2", [D, U], f32, kind="ExternalInput").ap()
    QT_d = nc.dram_tensor("QT", [D, TQ], f32, kind="ExternalInput").ap()
    VT_d = nc.dram_tensor("VT", [D, TV], f32, kind="ExternalInput").ap()
    VAL_d = nc.dram_tensor("VAL", [TV, D], f32, kind="ExternalInput").ap()
    VWH_d = nc.dram_tensor("VWH", [P, CU, 256], f32, kind="ExternalInput").ap()
    VWL_d = nc.dram_tensor("VWL", [P, CU, 256], f32, kind="ExternalInput").ap()
    B12_d = nc.dram_tensor("B12", [P, CU], f32, kind="ExternalInput").ap()
    ID_d = nc.dram_tensor("ID", [P, P], f32, kind="ExternalInput").ap()
    ME_d = nc.dram_tensor("ME", [P, 1], mybir.dt.uint8, kind="ExternalInput").ap()
    OUT_d = nc.dram_tensor("OUT", [TQ, D], f32, kind="ExternalOutput").ap()

    with tile.TileContext(nc) as tc:
        with (
            tc.tile_pool(name="cst", bufs=1) as cst,
            tc.tile_pool(name="pre_p", bufs=2) as pre_p,
            tc.tile_pool(name="feat_p", bufs=2) as feat_p,
            tc.tile_pool(name="sm", bufs=1) as sm,
            tc.tile_pool(name="ps", bufs=1, space=bass.MemorySpace.PSUM) as ps,
        ):
            # ---- constant / input tiles ----
            w1 = cst.tile([P, KD, U], f32, tag="w1")
            nc.sync.dma_start(w1[:], W1_d.rearrange("(k p) u -> p k u", p=P))
            w2 = cst.tile([P, KD, U], f32, tag="w2")
            nc.sync.dma_start(w2[:], W2_d.rearrange("(k p) u -> p k u", p=P))
            qt = cst.tile([P, KD, TQ], f32, tag="qt")
            nc.sync.dma_start(qt[:], QT_d.rearrange("(k p) q -> p k q", p=P))
            vt = cst.tile([P, KD, TV], f32, tag="vt")
            nc.sync.dma_start(vt[:], VT_d.rearrange("(k p) v -> p k v", p=P))
            val = cst.tile([P, CV, D], f32, tag="val")
            nc.sync.dma_start(val[:], VAL_d.rearrange("(c p) d -> p c d", p=P))
            f32r = mybir.dt.float32r
            vwh_f = cst.tile([P, CU, 256], f32, tag="vwh_f")
            nc.sync.dma_start(vwh_f[:], VWH_d)
            vwl_f = cst.tile([P, CU, 256], f32, tag="vwl_f")
            nc.sync.dma_start(vwl_f[:], VWL_d)
            vwh = cst.tile([P, CU, 256], f32r, tag="vwh")
            nc.vector.tensor_copy(vwh[:], vwh_f[:])
            vwl = cst.tile([P, CU, 256], f32r, tag="vwl")
            nc.vector.tensor_copy(vwl[:], vwl_f[:])
            b12 = cst.tile([P, CU], f32, tag="b12")
            nc.sync.dma_start(b12[:], B12_d)
            idt = cst.tile([P, P], f32, tag="idt")
            nc.sync.dma_start(idt[:], ID_d)
            mev = cst.tile([P, 1], mybir.dt.uint8, tag="mev")
            nc.sync.dma_start(mev[:], ME_d)

            # ---- projections ----
            psW1 = ps.tile([P, CU, TV], f32, tag="psW1")   # one bank
            for c in range(CU):
                for k in range(KD):
                    nc.tensor.matmul(psW1[:, c, :],
                                     w1[:, k, c * P:(c + 1) * P],
                                     vt[:, k, :],
                                     start=(k == 0), stop=(k == KD - 1))
            w1vT = cst.tile([P, CU, TV], f32, tag="w1vT")
            nc.scalar.copy(w1vT[:], psW1[:])

            psW2 = ps.tile([P, CU, TQ], f32, tag="psW2")   # half bank
            for c in range(CU):
                for k in range(KD):
                    nc.tensor.matmul(psW2[:, c, :],
                                     w2[:, k, c * P:(c + 1) * P],
                                     qt[:, k, :],
                                     start=(k == 0), stop=(k == KD - 1))
            w2qT = cst.tile([P, CU, TQ], f32, tag="w2qT")
            for c in range(CU):
                nc.scalar.activation(w2qT[:, c, :], psW2[:, c, :],
                                     AF.Identity, bias=b12[:, c:c + 1])

            # ---- score phase ----
            score_ps = ps.tile([P, 2 * TV], f32, tag="score")  # one bank
            n_mm = (TQ // 2) * CU * 2
            mm = 0
            for blk in range(NBLK):
                q0 = blk * BQ
                # layout [P, CU, BQ, TV]: keeps each pair's matmul rhs
                # (2 q's x 256 v within one chunk) contiguous
                pre = pre_p.tile([P, CU, BQ, TV], f32, tag="pre")
                feat = feat_p.tile([P, CU, BQ, TV], f32r, tag="feat")
                # DVE adds for q0 .. q0+N_DVE-1
                in0 = w1vT[:].unsqueeze(2).broadcast_to([P, CU, N_DVE, TV])
                in1 = (w2qT[:, :, q0:q0 + N_DVE]
                       .unsqueeze(3).broadcast_to([P, CU, N_DVE, TV]))
                nc.vector.tensor_add(pre[:, :, 0:N_DVE, :], in0, in1)
                # GPSIMD adds (per (q,c) tensor_scalar_add)
                for j in range(N_GP):
                    ql = N_DVE + j
                    q = q0 + ql
                    for c in range(CU):
                        nc.gpsimd.tensor_scalar_add(pre[:, c, ql, :],
                                                    w1vT[:, c, :],
                                                    w2qT[:, c, q:q + 1])
                # big tanh over the DVE+GPSIMD q's
                nt = N_DVE + N_GP
                nc.scalar.activation(feat[:, :, 0:nt, :],
                                     pre[:, :, 0:nt, :], AF.Tanh)
                # ACT-fused q's: feat = tanh(w1vT + w2q_col) directly
                for j in range(N_ACT):
                    ql = nt + j
                    q = q0 + ql
                    for c in range(CU):
                        nc.scalar.activation(feat[:, c, ql, :],
                                             psW1[:, c, :], AF.Tanh,
                                             bias=w2qT[:, c, q:q + 1])
                # score pair matmuls: pair p covers q=2p, 2p+1
                for pl in range(BQ // 2):
                    q = q0 + 2 * pl
                    for c in range(CU):
                        rhs = feat[:, c, 2 * pl:2 * pl + 2, :]
                        for w in (vwh, vwl):
                            nc.tensor.matmul(score_ps[:],
                                             w[:, c, 127 - q:255 - q],
                                             rhs,
                                             start=(mm == 0),
                                             stop=(mm == n_mm - 1))
                            mm += 1

            # ---- softmax (no max subtraction; scores are bounded) ----
            att_e = sm.tile([P, TV], f32, tag="att_e")
            att_o = sm.tile([P, TV], f32, tag="att_o")
            sums = sm.tile([P, 4], f32, tag="sums")
            nc.scalar.activation(att_e[:], score_ps[:, 0:TV], AF.Exp,
                                 accum_out=sums[:, 0:1])
            nc.scalar.activation(att_o[:], score_ps[:, TV:2 * TV], AF.Exp,
                                 accum_out=sums[:, 1:2])
            att = sm.tile([P, TV], f32, tag="att")
            nc.vector.tensor_copy(att[:], att_o[:])
            nc.vector.copy_predicated(att[:], mev[:].broadcast_to([P, TV]),
                                      att_e[:])
            nc.vector.tensor_copy(sums[:, 2:3], sums[:, 1:2])
            nc.vector.copy_predicated(sums[:, 2:3], mev[:], sums[:, 0:1])
            nc.vector.reciprocal(sums[:, 3:4], sums[:, 2:3])

            # ---- context = softmax(score) @ values ----
            psT = ps.tile([P, CV, P], f32, tag="psT")      # half bank
            for c in range(CV):
                nc.tensor.transpose(psT[:, c, :], att[:, c * P:(c + 1) * P],
                                    idt[:])
            attnT = sm.tile([P, CV, P], f32, tag="attnT")
            nc.scalar.copy(attnT[:], psT[:])

            ctx_ps = ps.tile([P, D], f32, tag="ctx")       # one bank
            for c in range(CV):
                nc.tensor.matmul(ctx_ps[:], attnT[:, c, :], val[:, c, :],
                                 start=(c == 0), stop=(c == CV - 1))
            ctx = sm.tile([P, D], f32, tag="ctxsb")
            nc.scalar.mul(ctx[:], ctx_ps[:], sums[:, 3:4])
            nc.sync.dma_start(OUT_d, ctx[:])

    nc.compile()
    return nc


def _tf32_rne(x):
    b = np.asarray(x, np.float32).view(np.uint32)
    b = (b + 0x7FF + ((b >> 12) & 1)) & np.uint32(0xFFFFF000)
    return b.view(np.float32)


def _prep_shared(W1, b1, W2, b2, V, bv):
    Vf = np.asarray(V, np.float32)[:, 0]
    Vh = _tf32_rne(Vf)
    Vl = _tf32_rne(Vf - Vh)
    Vwh = np.zeros((P, CU, 256), np.float32)
    Vwl = np.zeros((P, CU, 256), np.float32)
    for c in range(CU):
        Vwh[:, c, 127] = Vh[c * P:(c + 1) * P]
        Vwh[:, c, 128] = Vh[c * P:(c + 1) * P]
        Vwl[:, c, 127] = Vl[c * P:(c + 1) * P]
        Vwl[:, c, 128] = Vl[c * P:(c + 1) * P]
    b12 = (b1 + b2).astype(np.float32).reshape(CU, P).T.copy()
    ident = np.eye(P, dtype=np.float32)
    maskE = (1 - (np.arange(P) % 2)).astype(np.uint8).reshape(P, 1)
    return {
        "W1": np.ascontiguousarray(W1, np.float32),
        "W2": np.ascontiguousarray(W2, np.float32),
        "VWH": Vwh,
        "VWL": Vwl,
        "B12": np.ascontiguousarray(b12),
        "ID": ident,
        "ME": maskE,
    }


def kernel(query, values, W1, b1, W2, b2, V, bv, _trace=False, _tmpdir=None):
    global _compiled
    from concourse.bass_utils import run_bass_kernel_spmd

    query = np.asarray(query, np.float32)
    values = np.asarray(values, np.float32)
    shared = _prep_shared(np.asarray(W1), np.asarray(b1), np.asarray(W2),
                          np.asarray(b2), np.asarray(V), np.asarray(bv))

    if _compiled is None:
        _compiled = _build()
    nc = _compiled

    in_maps = []
    for i in range(B):
        m = dict(shared)
        m["QT"] = np.ascontiguousarray(query[i].T)
        m["VT"] = np.ascontiguousarray(values[i].T)
        m["VAL"] = np.ascontiguousarray(values[i])
        in_maps.append(m)

    kw = {}
    if _trace:
        kw.update(trace=True, tmpdir=_tmpdir)
    res = run_bass_kernel_spmd(nc, in_maps, core_ids=list(range(B)), **kw)
    out = np.stack([res.results[i]["OUT"] for i in range(B)], axis=0)
    if _trace:
        kernel._last_trace = res
    return out
